# revision 3
# baseline (speedup 1.0000x reference)
"""Trainium2 Bass kernel: fused bmm+decay+reduce attention scorer.

Computes, for full inputs
    self_attn  [N=16, M=100, EMB=128] f32
    self_delta [N=16, M=100, L=10000, D=4] f32
    emb_table  [L+1=10001, EMB=128] f32
    value_w    [M=100] f32
the output
    out[n, l] = sum_m value_w[m] * (sum_d self_delta[n,m,l,d]) * (emb_table[1+l] . self_attn[n,m])
of shape [16, 10000] f32 (matches the reference jnp einsum chain).

Sharding: the candidate/location axis L is split 8 ways (1250 locations per
core); every core handles all 16 batch rows for its location range.

v3, from trace analysis of the 83us baseline and the 70us v2:
  - self_delta is staged host-side as int8 (quant step 4*sigma/127, folded
    into the vwoh weights) and widened to bf16 *inside the DMA* (SWDGE cast):
    HBM-side bytes halve to 8 MB/core (~1.8us/tile at 358 GB/s) and the
    stream floor moves to the SBUF-AXI write side (1.28 MB/tile bf16 at
    435 GB/s ~ 2.9us/tile).  DVE still sees dense step-1 bf16 (2x mode).
    int8 quantization adds ~0.9% rms error vs the 2e-2 budget (bf16 total
    measured 4.6e-3).
  - The PE's HAM clock gate re-evaluates on free-running 3413ns windows and
    needs roughly >62% duty to hold K=8/8 (measured: a 62%-duty warm phase
    re-throttled, the cold saturated phase never re-warmed).  Warm-up MMs
    bridge the preamble->first-tile gap, and 3 small filler MMs per tile
    pad steady-state duty to ~90% so the stream holds 2.4 GHz.
  - Constants travel as ONE SWDGE DMA (concatenated [128, x] tensor): the
    HWDGE rings issue descriptors at ~22ns each, so a 432-descriptor
    constant load costs 9.5us there (v2 measured) but ~2us via the Q7
    CounterMachine.
  - The decay multiply is split DVE/GpSimd (768/482 columns) so DVE's
    fold1+fold2+mul chain (2.8us) fits inside the 2.9us AXI tile period.
  - Output: per-chunk PSUM copy + HWDGE (sync) DMA interleaved with the
    final acc matmuls.
Fixed framework overhead observed in every NEFF execution: ~6.5us engine
preamble + ~7.5us trailing next-iteration preamble (253 serial semaphore
resets) — not addressable from kernel code.
"""

import ml_dtypes
import numpy as np

import concourse.mybir as mybir
import concourse.tile as tile
from concourse import bacc
from concourse.bass_utils import run_bass_kernel_spmd

BF16 = ml_dtypes.bfloat16

N, M, L, EMB, D = 16, 100, 10000, 128, 4
NCORES = 8
LSH = L // NCORES  # 1250 locations per core
R = N * M  # 1600 flattened (n, m) rows
P = 128
NTILE = (R + P - 1) // P  # 13 tiles; the last holds 64 real rows
ROW0 = [t * P for t in range(NTILE - 1)] + [R - 64]
TILE_ORDER = list(range(NTILE))  # remainder tile last
CHUNKS = [(0, 512), (512, 512), (1024, 226)]
HALF = LSH // 2  # 625
DT16 = mybir.dt.bfloat16
INT8 = mybir.dt.int8
FP32 = mybir.dt.float32
N_WARM = 7  # warm-up matmuls (N=512) bridging preamble -> first S matmul
N_FILL = 3  # keep-warm filler matmuls (N=256) per tile
MULSPLIT = 768  # decay-mul columns on DVE; the rest go to GpSimd
# const concat layout (bf16 columns): embT | attnT | vwoh
C_EMB0, C_ATT0, C_VW0 = 0, LSH, LSH + R
C_W = LSH + R + NTILE * N

_NC_CACHE = {}


def _build_nc():
    nc = bacc.Bacc(
        "TRN2", target_bir_lowering=False, debug=False, num_devices=NCORES
    )
    # raw8[r, d*LSH + l] = int8 quantized self_delta[n, m, lo+l, d]
    raw_d = nc.dram_tensor("raw", [R, D * LSH], INT8, kind="ExternalInput").ap()
    cst_d = nc.dram_tensor("cst", [P, C_W], DT16, kind="ExternalInput").ap()
    out_d = nc.dram_tensor("out", [N, LSH], FP32, kind="ExternalOutput").ap()

    with tile.TileContext(nc) as tc:
        with (
            tc.tile_pool(name="const", bufs=1) as cpool,
            tc.tile_pool(name="raws", bufs=6) as rpool,
            tc.tile_pool(name="a1p", bufs=2) as a1pool,
            tc.tile_pool(name="work", bufs=2) as wpool,
            tc.tile_pool(name="spsum", bufs=1, space="PSUM") as spool,
            tc.tile_pool(name="apsum", bufs=1, space="PSUM") as apool,
            tc.tile_pool(name="wpsum", bufs=1, space="PSUM") as wmpool,
        ):
            # warm-up operand via on-chip memset: no DMA dependency
            warm = cpool.tile([P, 512], DT16, tag="warm")
            nc.vector.memset(warm, 0.001)
            wps = wmpool.tile([P, 512], FP32, tag="wps")
            for _ in range(N_WARM):
                nc.tensor.matmul(wps, warm[:, 0:P], warm, start=True, stop=True)

            # all constants in one SWDGE DMA (fast Q7 descriptor generation)
            cst = cpool.tile([P, C_W], DT16, tag="cst")
            nc.gpsimd.dma_start(out=cst, in_=cst_d)
            embT = cst[:, C_EMB0:C_ATT0]
            attnT = cst[:, C_ATT0:C_VW0]
            vwoh = cst[:, C_VW0:C_W]

            acc = apool.tile([N, LSH], FP32, tag="acc")

            pending = None  # (pt, t) of the previous tile, acc-mm'd next iter

            def emit_acc(pt, t, *, first, last):
                for c0, w in CHUNKS:
                    nc.tensor.matmul(
                        acc[:, c0 : c0 + w],
                        vwoh[:, t * N : (t + 1) * N],
                        pt[:, c0 : c0 + w],
                        start=first,
                        stop=last,
                    )

            for ti, t in enumerate(TILE_ORDER):
                last = ti == NTILE - 1
                raw = rpool.tile([P, D * LSH], DT16, tag="raw")
                if last:
                    # only the 64 remainder rows, staged half/quarter/quarter
                    rv = raw.rearrange("p (d l) -> p d l", d=D)[0:64]
                    rd3 = raw_d.rearrange("r (d l) -> r d l", d=D)
                    rr = slice(ROW0[t], ROW0[t] + 64)
                    nc.gpsimd.dma_start(out=rv[:, 0:2], in_=rd3[rr, 0:2])
                    nc.gpsimd.dma_start(
                        out=rv[:, 2:4, 0:HALF], in_=rd3[rr, 2:4, 0:HALF]
                    )
                    nc.gpsimd.dma_start(
                        out=rv[:, 2:4, HALF:LSH], in_=rd3[rr, 2:4, HALF:LSH]
                    )
                elif ti == 0:
                    # first tile as two half DMAs: the d-fold starts earlier
                    nc.gpsimd.dma_start(
                        out=raw[:, 0 : 2 * LSH],
                        in_=raw_d[ROW0[t] : ROW0[t] + P, 0 : 2 * LSH],
                    )
                    nc.gpsimd.dma_start(
                        out=raw[:, 2 * LSH : 4 * LSH],
                        in_=raw_d[ROW0[t] : ROW0[t] + P, 2 * LSH : 4 * LSH],
                    )
                else:
                    nc.gpsimd.dma_start(out=raw, in_=raw_d[ROW0[t] : ROW0[t] + P])

                # S[p, l] = attn[r(p)] . emb_cand[lo+l], bf16 in, fp32 PSUM out
                rows = 64 if last else P
                s_ps = spool.tile([P, LSH], FP32, tag="s")
                for c0, w in CHUNKS:
                    nc.tensor.matmul(
                        s_ps[:rows, c0 : c0 + w],
                        attnT[:, ROW0[t] : ROW0[t] + rows],
                        embT[:, c0 : c0 + w],
                        start=True,
                        stop=True,
                    )
                # previous tile's output accumulation rides behind this
                # tile's S matmuls in the PE stream
                if pending is not None:
                    emit_acc(*pending, first=(ti == 1), last=False)
                # keep-warm fillers: pad PE duty over the HAM window threshold
                for _ in range(N_FILL):
                    nc.tensor.matmul(
                        wps[:, 0:256], warm[:, 0:P], warm[:, 0:256],
                        start=True, stop=True,
                    )

                # ScalarE evacuates S to SBUF as bf16 (2x-mode DVE operand)
                s_sb = wpool.tile([P, LSH], DT16, tag="ssb")
                nc.scalar.copy(out=s_sb, in_=s_ps)

                # delta[p,l] = sum_d raw[p,l,d]: dense 2x-mode adds
                a1 = a1pool.tile([P, 2 * LSH], DT16, tag="a1")
                a2 = wpool.tile([P, LSH], DT16, tag="a2")
                pt = wpool.tile([P, LSH], DT16, tag="pt")
                if last:
                    nc.vector.tensor_add(
                        out=a1[:, 0:LSH], in0=raw[:, 0:LSH], in1=raw[:, LSH : 2 * LSH]
                    )
                    for h0, h1 in ((0, HALF), (HALF, LSH)):
                        nc.vector.tensor_add(
                            out=a1[:, LSH + h0 : LSH + h1],
                            in0=raw[:, 2 * LSH + h0 : 2 * LSH + h1],
                            in1=raw[:, 3 * LSH + h0 : 3 * LSH + h1],
                        )
                        nc.vector.tensor_add(
                            out=a2[:, h0:h1],
                            in0=a1[:, h0:h1],
                            in1=a1[:, LSH + h0 : LSH + h1],
                        )
                        nc.vector.tensor_mul(
                            out=pt[:, h0:h1], in0=a2[:, h0:h1], in1=s_sb[:, h0:h1]
                        )
                elif ti == 0:
                    nc.vector.tensor_add(
                        out=a1[:, 0:LSH], in0=raw[:, 0:LSH], in1=raw[:, LSH : 2 * LSH]
                    )
                    nc.vector.tensor_add(
                        out=a1[:, LSH : 2 * LSH],
                        in0=raw[:, 2 * LSH : 3 * LSH],
                        in1=raw[:, 3 * LSH : 4 * LSH],
                    )
                    nc.vector.tensor_add(
                        out=a2, in0=a1[:, 0:LSH], in1=a1[:, LSH : 2 * LSH]
                    )
                else:
                    nc.vector.tensor_add(
                        out=a1, in0=raw[:, 0 : 2 * LSH], in1=raw[:, 2 * LSH : 4 * LSH]
                    )
                    nc.vector.tensor_add(
                        out=a2, in0=a1[:, 0:LSH], in1=a1[:, LSH : 2 * LSH]
                    )
                if not last:
                    # decay multiply, split DVE / GpSimd to fit the AXI pace
                    nc.vector.tensor_mul(
                        out=pt[:, 0:MULSPLIT],
                        in0=a2[:, 0:MULSPLIT],
                        in1=s_sb[:, 0:MULSPLIT],
                    )
                    nc.gpsimd.tensor_mul(
                        out=pt[:, MULSPLIT:LSH],
                        in0=a2[:, MULSPLIT:LSH],
                        in1=s_sb[:, MULSPLIT:LSH],
                    )
                pending = (pt, t)

            # final tile's acc matmuls, interleaved per chunk with the PSUM
            # evacuation copy and a per-chunk output DMA
            pt, t = pending
            out_sb = cpool.tile([N, LSH], FP32, tag="out_sb")
            for c0, w in CHUNKS:
                nc.tensor.matmul(
                    acc[:, c0 : c0 + w],
                    vwoh[:, t * N : (t + 1) * N],
                    pt[:, c0 : c0 + w],
                    start=False,
                    stop=True,
                )
                nc.vector.tensor_copy(
                    out=out_sb[:, c0 : c0 + w], in_=acc[:, c0 : c0 + w]
                )
                nc.sync.dma_start(
                    out=out_d[:, c0 : c0 + w], in_=out_sb[:, c0 : c0 + w]
                )

    nc.compile()
    return nc


def _get_nc():
    if "nc" not in _NC_CACHE:
        _NC_CACHE["nc"] = _build_nc()
    return _NC_CACHE["nc"]


def _prep_in_maps(self_attn, self_delta, emb_table, value_w):
    self_attn = np.asarray(self_attn, dtype=np.float32)
    self_delta = np.asarray(self_delta, dtype=np.float32)
    emb_table = np.asarray(emb_table, dtype=np.float32)
    value_w = np.asarray(value_w, dtype=np.float32)

    # int8 quantization of the delta stream (device DMA widens to bf16)
    qs = 4.0 * float(self_delta.std()) / 127.0
    raw_q = np.clip(np.rint(self_delta * (1.0 / qs)), -127, 127).astype(np.int8)

    embT_full = emb_table[1 : L + 1].T.astype(BF16)  # [EMB, L]
    attnT = self_attn.transpose(2, 0, 1).reshape(EMB, R).astype(BF16)

    # vwoh[p, t*N + j] = qs * vw[m(r)] * (n(r) == j),  r = ROW0[t] + p; the
    # quant step is folded in here.  The overlap tile's first 64 rows are
    # already counted by tile 11 -> zero.
    vwoh = np.zeros((P, NTILE * N), dtype=BF16)
    for t in range(NTILE):
        pmax = 64 if t == NTILE - 1 else P
        for p in range(pmax):
            r = ROW0[t] + p
            vwoh[p, t * N + (r // M)] = qs * value_w[r % M]

    in_maps = []
    for c in range(NCORES):
        lo = c * LSH
        # raw[r, d*LSH + l] = q(self_delta[n, m, lo+l, d])
        raw_c = np.empty((R, D * LSH), dtype=np.int8)
        raw_c.reshape(N, M, D, LSH)[...] = raw_q[:, :, lo : lo + LSH, :].transpose(
            0, 1, 3, 2
        )
        cst = np.empty((P, C_W), dtype=BF16)
        cst[:, C_EMB0:C_ATT0] = embT_full[:, lo : lo + LSH]
        cst[:, C_ATT0:C_VW0] = attnT
        cst[:, C_VW0:C_W] = vwoh
        in_maps.append({"raw": raw_c, "cst": cst})
    return in_maps


def _run(inputs, **spmd_kwargs):
    in_maps = _prep_in_maps(
        inputs["self_attn"], inputs["self_delta"], inputs["emb_table"], inputs["value_w"]
    )
    res = run_bass_kernel_spmd(
        _get_nc(), in_maps, core_ids=list(range(NCORES)), **spmd_kwargs
    )
    out = np.concatenate([r["out"] for r in res.results], axis=1)  # [N, L]
    return out, res


def kernel(**inputs) -> np.ndarray:
    out, _ = _run(inputs)
    return out


# revision 7
# speedup vs baseline: 1.1437x; 1.1437x over previous
"""Trainium2 Bass kernel: fused bmm+decay+reduce attention scorer.

Computes, for full inputs
    self_attn  [N=16, M=100, EMB=128] f32
    self_delta [N=16, M=100, L=10000, D=4] f32
    emb_table  [L+1=10001, EMB=128] f32
    value_w    [M=100] f32
the output
    out[n, l] = sum_m value_w[m] * (sum_d self_delta[n,m,l,d]) * (emb_table[1+l] . self_attn[n,m])
of shape [16, 10000] f32 (matches the reference jnp einsum chain).

Sharding: the candidate/location axis L is split 8 ways (1250 locations per
core); every core handles all 16 batch rows for its location range.

v4, after tracing v2 (70us) and v3 (89us):
  - self_delta staged host-side as int8 (quant step 4*sigma/127 folded into
    vwoh) and widened to bf16 inside the DMA (SWDGE cast): HBM bytes halve
    to 8 MB/core; the stream floor moves to the SBUF-AXI write side
    (~2.9us per 128-row tile at 435 GB/s).  DVE still sees step-1 bf16.
  - Every DVE instruction pays a ~270ns pipeline DRAIN, so the 3-op
    fold/fold/mul chain really costs 3.7us per tile (measured 1744/974/974)
    — that, not the PE, paced v2.  Tiles are processed in PAIRS living in
    one SBUF slot so each DVE op covers two tiles (one drain), and the
    decay multiply is split DVE(832 cols/tile)/GpSimd(418): DVE lands at
    ~2.9us/tile, matching the AXI pace.
  - v3's fatal mistake: per-tile GpSimd muls were emitted BEFORE later
    tiles' SWDGE triggers in GpSimd's strict-FIFO queue, serializing the
    whole stream behind the compute chain.  v4 emits ALL stream triggers
    up-front (slots are distinct buffers, no waits); only the remainder's
    staged DMAs (which reuse slot 0) are emitted mid-loop, after pair 1.
  - PE HAM clock gate: warm (2.4 GHz) needs roughly >62% duty per 3413ns
    window.  Warm-up MMs (into the acc bank, junk cleared by the real
    accumulation's start=True) bridge the preamble; LDWEIGHTS-only fillers
    pad duty in the steady state without touching PSUM.
  - Constants travel as ONE SWDGE DMA (HWDGE rings issue descriptors at
    ~22ns each — a 432-descriptor constant load measured 9.5us there).
  - Output: per-chunk PSUM copy + sync-HWDGE DMA interleaved with the
    final acc matmuls.
Fixed framework overhead in every NEFF execution: ~6.5us engine preamble +
~7.5us trailing next-iteration preamble (253 serial semaphore resets).
"""

import ml_dtypes
import numpy as np

import concourse.mybir as mybir
import concourse.tile as tile
from concourse import bacc
from concourse.bass_utils import run_bass_kernel_spmd

BF16 = ml_dtypes.bfloat16

N, M, L, EMB, D = 16, 100, 10000, 128, 4
NCORES = 8
LSH = L // NCORES  # 1250 locations per core
R = N * M  # 1600 flattened (n, m) rows
P = 128
NTILE = 13  # 12 full 128-row tiles (6 pairs) + one 64-row remainder
ROW0 = [t * P for t in range(NTILE - 1)] + [R - 64]
NPAIR = 6
CHUNKS = [(0, 512), (512, 512), (1024, 226)]  # acc banks / tile-0-of-pair S
CHUNKS1 = [(1250, 286), (1536, 512), (2048, 452)]  # tile-1-of-pair S chunks
HALF = LSH // 2  # 625
W = D * LSH  # 5000 columns per tile in the raw stream
DT16 = mybir.dt.bfloat16
INT8 = mybir.dt.int8
FP32 = mybir.dt.float32
N_WARM = 7  # warm-up matmuls (N=512) bridging preamble -> first S matmul
N_FILL = 3  # keep-warm LDWEIGHTS fillers per pair
MS = 832  # decay-mul columns per tile on DVE; the rest (418) on GpSimd
# const concat layout (bf16 columns): embT | attnT | vwoh
C_EMB0, C_ATT0, C_VW0 = 0, LSH, LSH + R
C_W = LSH + R + NTILE * N

_NC_CACHE = {}


def _build_nc():
    nc = bacc.Bacc(
        "TRN2", target_bir_lowering=False, debug=False, num_devices=NCORES
    )
    # raw8[r, d*LSH + l] = int8 quantized self_delta[n, m, lo+l, d]
    raw_d = nc.dram_tensor("raw", [R, W], INT8, kind="ExternalInput").ap()
    cst_d = nc.dram_tensor("cst", [P, C_W], DT16, kind="ExternalInput").ap()
    out_d = nc.dram_tensor("out", [N, LSH], FP32, kind="ExternalOutput").ap()

    with tile.TileContext(nc) as tc:
        with (
            tc.tile_pool(name="const", bufs=1) as cpool,
            tc.tile_pool(name="raws", bufs=6) as rpool,
            tc.tile_pool(name="a1p", bufs=2) as a1pool,
            tc.tile_pool(name="work", bufs=2) as wpool,
            tc.tile_pool(name="spsum", bufs=1, space="PSUM") as spool,
            tc.tile_pool(name="apsum", bufs=1, space="PSUM") as apool,
        ):
            # out accumulator rows n=0..15, 3 PSUM banks, lives whole kernel
            acc = apool.tile([N, LSH], FP32, tag="acc")

            # PE warm-up: operand memset on-chip (no DMA dependency); junk
            # output goes to the acc bank and is cleared by the real
            # accumulation's start=True later.
            warm = cpool.tile([P, 512], DT16, tag="warm")
            nc.vector.memset(warm, 0.001)
            for _ in range(N_WARM):
                nc.tensor.matmul(
                    acc[:, 0:512], warm[:, 0:N], warm, start=True, stop=True
                )

            # all constants in one SWDGE DMA (fast Q7 descriptor generation)
            cst = cpool.tile([P, C_W], DT16, tag="cst")
            nc.gpsimd.dma_start(out=cst, in_=cst_d)
            embT = cst[:, C_EMB0:C_ATT0]
            attnT = cst[:, C_ATT0:C_VW0]
            vwoh = cst[:, C_VW0:C_W]

            # --- stream DMAs, ALL triggered up-front (distinct slots: the
            # GpSimd FIFO never blocks on a buffer-free wait here)
            pairs = []  # per pair: raw tile [P, 2*W]
            for p in range(NPAIR):
                raw = rpool.tile([P, 2 * W], DT16, tag="raw")
                r0 = ROW0[2 * p]
                if p == 0:
                    # tile 0 in two halves so the d-fold starts earlier
                    nc.gpsimd.dma_start(
                        out=raw[:, 0 : 2 * LSH], in_=raw_d[r0 : r0 + P, 0 : 2 * LSH]
                    )
                    nc.gpsimd.dma_start(
                        out=raw[:, 2 * LSH : W], in_=raw_d[r0 : r0 + P, 2 * LSH : W]
                    )
                else:
                    nc.gpsimd.dma_start(out=raw[:, 0:W], in_=raw_d[r0 : r0 + P])
                nc.gpsimd.dma_start(
                    out=raw[:, W : 2 * W], in_=raw_d[r0 + P : r0 + 2 * P]
                )
                pairs.append(raw)

            def emit_S(t, s_ps, chunks, rows=P):
                # S[p, l] = attn[r(p)] . emb_cand[lo+l] into the pair PSUM
                base = chunks[0][0]
                for c0, w in chunks:
                    nc.tensor.matmul(
                        s_ps[:rows, c0 : c0 + w],
                        attnT[:, ROW0[t] : ROW0[t] + rows],
                        embT[:, c0 - base : c0 - base + w],
                        start=True,
                        stop=True,
                    )

            def emit_acc(pt, u, v, *, first, last):
                # pt holds two tiles' products at cols [0:1250] / [1250:2500]
                for half, t in ((0, u), (1, v)):
                    for c0, w in CHUNKS:
                        nc.tensor.matmul(
                            acc[:, c0 : c0 + w],
                            vwoh[:, t * N : (t + 1) * N],
                            pt[:, half * LSH + c0 : half * LSH + c0 + w],
                            start=(first and half == 0),
                            stop=last,
                        )

            pending = None  # (pt, u, v) of the previous pair

            for p in range(NPAIR):
                raw = pairs[p]
                u, v = 2 * p, 2 * p + 1
                s_ps = spool.tile([P, 2 * LSH], FP32, tag="s")  # 5 banks
                emit_S(u, s_ps, CHUNKS)
                emit_S(v, s_ps, CHUNKS1)
                if pending is not None:
                    emit_acc(*pending, first=(p == 1), last=False)
                for _ in range(N_FILL):
                    nc.tensor.ldweights(weights=warm[:, 0:P])

                s_sb = wpool.tile([P, 2 * LSH], DT16, tag="ssb")
                a1 = a1pool.tile([P, 2 * 2 * LSH], DT16, tag="a1")
                a2 = wpool.tile([P, 2 * LSH], DT16, tag="a2")
                pt = wpool.tile([P, 2 * LSH], DT16, tag="pt")

                if p == 0:
                    # per-tile ops: tile 0's fold can start after its first
                    # half DMA; everything on DVE during the ramp
                    nc.scalar.copy(out=s_sb[:, 0:LSH], in_=s_ps[:, 0:LSH])
                    nc.scalar.copy(out=s_sb[:, LSH : 2 * LSH], in_=s_ps[:, LSH : 2 * LSH])
                    for ti in range(2):
                        r0c = ti * W
                        a0c = ti * 2 * LSH
                        sc = ti * LSH
                        if ti == 0:
                            nc.vector.tensor_add(
                                out=a1[:, a0c : a0c + LSH],
                                in0=raw[:, r0c : r0c + LSH],
                                in1=raw[:, r0c + LSH : r0c + 2 * LSH],
                            )
                            nc.vector.tensor_add(
                                out=a1[:, a0c + LSH : a0c + 2 * LSH],
                                in0=raw[:, r0c + 2 * LSH : r0c + 3 * LSH],
                                in1=raw[:, r0c + 3 * LSH : r0c + 4 * LSH],
                            )
                        else:
                            nc.vector.tensor_add(
                                out=a1[:, a0c : a0c + 2 * LSH],
                                in0=raw[:, r0c : r0c + 2 * LSH],
                                in1=raw[:, r0c + 2 * LSH : r0c + 4 * LSH],
                            )
                        nc.vector.tensor_add(
                            out=a2[:, sc : sc + LSH],
                            in0=a1[:, a0c : a0c + LSH],
                            in1=a1[:, a0c + LSH : a0c + 2 * LSH],
                        )
                        nc.vector.tensor_mul(
                            out=pt[:, sc : sc + LSH],
                            in0=a2[:, sc : sc + LSH],
                            in1=s_sb[:, sc : sc + LSH],
                        )
                else:
                    # pair-wide ops: one DRAIN each instead of two
                    nc.scalar.copy(out=s_sb, in_=s_ps)
                    rr = raw.rearrange("p (t h c) -> p t h c", t=2, h=2)
                    ar = a1.rearrange("p (t c) -> p t c", t=2)
                    nc.vector.tensor_add(
                        out=ar, in0=rr[:, :, 0], in1=rr[:, :, 1]
                    )
                    a2r = a2.rearrange("p (t c) -> p t c", t=2)
                    nc.vector.tensor_add(
                        out=a2r, in0=ar[:, :, 0:LSH], in1=ar[:, :, LSH : 2 * LSH]
                    )
                    ptr = pt.rearrange("p (t c) -> p t c", t=2)
                    sr = s_sb.rearrange("p (t c) -> p t c", t=2)
                    nc.vector.tensor_mul(
                        out=ptr[:, :, 0:MS], in0=a2r[:, :, 0:MS], in1=sr[:, :, 0:MS]
                    )
                    nc.gpsimd.tensor_mul(
                        out=ptr[:, :, MS:LSH],
                        in0=a2r[:, :, MS:LSH],
                        in1=sr[:, :, MS:LSH],
                    )
                pending = (pt, u, v)

                if p == 2:
                    # remainder DMAs reuse slot 0: by now pair 0's raw is
                    # consumed, so these never stall the GpSimd FIFO
                    rem = rpool.tile([P, 2 * W], DT16, tag="raw")
                    rv = rem[:, 0:W].rearrange("p (d l) -> p d l", d=D)[0:64]
                    rd3 = raw_d.rearrange("r (d l) -> r d l", d=D)
                    rr_ = slice(ROW0[12], ROW0[12] + 64)
                    nc.gpsimd.dma_start(out=rv[:, 0:2], in_=rd3[rr_, 0:2])
                    nc.gpsimd.dma_start(
                        out=rv[:, 2:4, 0:HALF], in_=rd3[rr_, 2:4, 0:HALF]
                    )
                    nc.gpsimd.dma_start(
                        out=rv[:, 2:4, HALF:LSH], in_=rd3[rr_, 2:4, HALF:LSH]
                    )

            # ---- remainder tile (64 rows), staged like the baseline
            t = 12
            raw = rem
            s_ps = spool.tile([P, 2 * LSH], FP32, tag="s")
            emit_S(t, s_ps, CHUNKS, rows=64)
            emit_acc(*pending, first=False, last=False)

            s_sb_f = wpool.tile([P, 2 * LSH], DT16, tag="ssb")
            a1_f = a1pool.tile([P, 2 * 2 * LSH], DT16, tag="a1")
            a2_f = wpool.tile([P, 2 * LSH], DT16, tag="a2")
            pt_f = wpool.tile([P, 2 * LSH], DT16, tag="pt")
            s_sb = s_sb_f[:, 0:LSH]
            a1 = a1_f[:, 0 : 2 * LSH]
            a2 = a2_f[:, 0:LSH]
            pt = pt_f[:, 0:LSH]
            nc.scalar.copy(out=s_sb, in_=s_ps[:, 0:LSH])
            nc.vector.tensor_add(
                out=a1[:, 0:LSH], in0=raw[:, 0:LSH], in1=raw[:, LSH : 2 * LSH]
            )
            for h0, h1 in ((0, HALF), (HALF, LSH)):
                nc.vector.tensor_add(
                    out=a1[:, LSH + h0 : LSH + h1],
                    in0=raw[:, 2 * LSH + h0 : 2 * LSH + h1],
                    in1=raw[:, 3 * LSH + h0 : 3 * LSH + h1],
                )
                nc.vector.tensor_add(
                    out=a2[:, h0:h1],
                    in0=a1[:, h0:h1],
                    in1=a1[:, LSH + h0 : LSH + h1],
                )
                nc.vector.tensor_mul(
                    out=pt[:, h0:h1], in0=a2[:, h0:h1], in1=s_sb[:, h0:h1]
                )

            # final acc matmuls interleaved per chunk with the PSUM copy and
            # a per-chunk output DMA on the sync HWDGE ring
            out_sb = cpool.tile([N, LSH], FP32, tag="out_sb")
            for c0, w in CHUNKS:
                nc.tensor.matmul(
                    acc[:, c0 : c0 + w],
                    vwoh[:, t * N : (t + 1) * N],
                    pt[:, c0 : c0 + w],
                    start=False,
                    stop=True,
                )
                nc.vector.tensor_copy(
                    out=out_sb[:, c0 : c0 + w], in_=acc[:, c0 : c0 + w]
                )
                nc.sync.dma_start(
                    out=out_d[:, c0 : c0 + w], in_=out_sb[:, c0 : c0 + w]
                )

    nc.compile()
    return nc


def _get_nc():
    if "nc" not in _NC_CACHE:
        _NC_CACHE["nc"] = _build_nc()
    return _NC_CACHE["nc"]


def _prep_in_maps(self_attn, self_delta, emb_table, value_w):
    self_attn = np.asarray(self_attn, dtype=np.float32)
    self_delta = np.asarray(self_delta, dtype=np.float32)
    emb_table = np.asarray(emb_table, dtype=np.float32)
    value_w = np.asarray(value_w, dtype=np.float32)

    # int8 quantization of the delta stream (device DMA widens to bf16)
    qs = 4.0 * float(self_delta.std()) / 127.0
    raw_q = np.clip(np.rint(self_delta * (1.0 / qs)), -127, 127).astype(np.int8)

    embT_full = emb_table[1 : L + 1].T.astype(BF16)  # [EMB, L]
    attnT = self_attn.transpose(2, 0, 1).reshape(EMB, R).astype(BF16)

    # vwoh[p, t*N + j] = qs * vw[m(r)] * (n(r) == j),  r = ROW0[t] + p; the
    # quant step is folded in here.  The overlap tile's first 64 rows are
    # already counted by tile 11 -> zero.
    vwoh = np.zeros((P, NTILE * N), dtype=BF16)
    for t in range(NTILE):
        pmax = 64 if t == NTILE - 1 else P
        for p in range(pmax):
            r = ROW0[t] + p
            vwoh[p, t * N + (r // M)] = qs * value_w[r % M]

    in_maps = []
    for c in range(NCORES):
        lo = c * LSH
        raw_c = np.empty((R, W), dtype=np.int8)
        raw_c.reshape(N, M, D, LSH)[...] = raw_q[:, :, lo : lo + LSH, :].transpose(
            0, 1, 3, 2
        )
        cst = np.empty((P, C_W), dtype=BF16)
        cst[:, C_EMB0:C_ATT0] = embT_full[:, lo : lo + LSH]
        cst[:, C_ATT0:C_VW0] = attnT
        cst[:, C_VW0:C_W] = vwoh
        in_maps.append({"raw": raw_c, "cst": cst})
    return in_maps


def _run(inputs, **spmd_kwargs):
    in_maps = _prep_in_maps(
        inputs["self_attn"], inputs["self_delta"], inputs["emb_table"], inputs["value_w"]
    )
    res = run_bass_kernel_spmd(
        _get_nc(), in_maps, core_ids=list(range(NCORES)), **spmd_kwargs
    )
    out = np.concatenate([r["out"] for r in res.results], axis=1)  # [N, L]
    return out, res


def kernel(**inputs) -> np.ndarray:
    out, _ = _run(inputs)
    return out


# revision 8
# speedup vs baseline: 1.1984x; 1.0479x over previous
"""Trainium2 Bass kernel: fused bmm+decay+reduce attention scorer.

Computes, for full inputs
    self_attn  [N=16, M=100, EMB=128] f32
    self_delta [N=16, M=100, L=10000, D=4] f32
    emb_table  [L+1=10001, EMB=128] f32
    value_w    [M=100] f32
the output
    out[n, l] = sum_m value_w[m] * (sum_d self_delta[n,m,l,d]) * (emb_table[1+l] . self_attn[n,m])
of shape [16, 10000] f32 (matches the reference jnp einsum chain).

Sharding: the candidate/location axis L is split 8 ways (1250 locations per
core); every core handles all 16 batch rows for its location range.

v5.  Measured foundations (from tracing v0=83us, v2=70us, v3=89us, v4=77us):
  - self_delta staged host-side as int8 (quant step 4*sigma/127 folded into
    vwoh) and widened to bf16 inside the SWDGE cast-DMA.  Measured: the
    cast stream runs at 425 GB/s on the SBUF-AXI write side -> floor
    ~6.0us per 256-row pair (HBM side is half the bytes, not binding).
  - DVE op cost = (58 + FD/accel)/0.96GHz + ~100ns drain.  Tiles are
    processed in PAIRS sharing one SBUF slot so each fold/mul op covers
    two tiles: a1(FD5000)+a2(FD2500)+mul(FD2500) ~ 5.7us/pair < pace.
  - GpSimd tensor ops BLOCK the DVE cycle-for-cycle (shared SBUF port pair;
    v4's gpsimd mul inflated overlapping DVE ops by exactly the overlap),
    so GpSimd does descriptor generation only, all elementwise on DVE.
  - GpSimd's NX is strict FIFO: all stream triggers are emitted up-front
    (distinct slots, no buffer-reuse waits except the late remainder).
  - PE HAM: K=8/8 needs >~62-90%% duty per free-running 3413ns window (62%%
    measured insufficient).  Warm-up MMs bridge the preamble; per-pair
    filler MMs whose moving operand is the previous pair's pt (a real
    dependency, so the scheduler cannot hoist them out of the steady
    state) pad duty.  Their output goes to the padding tail of the acc
    PSUM bank with start=False so accumulation bits are untouched.
  - ACTIVATE evacuation split [0:1024] (banks 0-1, issued after tile-u's
    S) and [1024:2500] (banks 2-4, after tile-v): the next pair's S
    matmuls wait only on the second part -> shorter serial chain, no PSUM
    read/write bank overlap at any point.
  - Per-chunk output copy + sync-HWDGE DMA interleaved with final accs.
Fixed framework overhead per NEFF execution: ~6.5us preamble + ~7.5us
trailing next-iteration preamble (253 serial semaphore resets).
"""

import ml_dtypes
import numpy as np

import concourse.mybir as mybir
import concourse.tile as tile
from concourse import bacc
from concourse.bass_utils import run_bass_kernel_spmd

BF16 = ml_dtypes.bfloat16

N, M, L, EMB, D = 16, 100, 10000, 128, 4
NCORES = 8
LSH = L // NCORES  # 1250 locations per core
R = N * M  # 1600 flattened (n, m) rows
P = 128
NTILE = 13  # 12 full 128-row tiles (6 pairs) + one 64-row remainder
ROW0 = [t * P for t in range(NTILE - 1)] + [R - 64]
NPAIR = 6
CHUNKS = [(0, 512), (512, 512), (1024, 226)]  # acc banks / tile-0-of-pair S
CHUNKS1 = [(1250, 286), (1536, 512), (2048, 452)]  # tile-1-of-pair S chunks
HALF = LSH // 2  # 625
W = D * LSH  # 5000 columns per tile in the raw stream
ACC_PAD = 1536  # acc PSUM tile padded to 3 full banks; tail = filler target
DT16 = mybir.dt.bfloat16
INT8 = mybir.dt.int8
FP32 = mybir.dt.float32
N_WARM = 7  # warm-up matmuls (N=512) bridging preamble -> first S matmul
N_FILL = 4  # keep-warm filler matmuls (N=256) per pair
# const concat layout (bf16 columns): embT | vwoh | attnT (attnT last so the
# hot prefix covering pairs 0..1 can land first)
C_EMB0 = 0
C_VW0 = LSH
C_ATT0 = LSH + NTILE * N
C_W = C_ATT0 + R
C_HOT = C_ATT0 + 4 * P  # embT + vwoh + attnT rows 0..511 (pairs 0-1)

_NC_CACHE = {}


def _build_nc():
    nc = bacc.Bacc(
        "TRN2", target_bir_lowering=False, debug=False, num_devices=NCORES
    )
    # raw8[r, d*LSH + l] = int8 quantized self_delta[n, m, lo+l, d]
    raw_d = nc.dram_tensor("raw", [R, W], INT8, kind="ExternalInput").ap()
    cst_d = nc.dram_tensor("cst", [P, C_W], DT16, kind="ExternalInput").ap()
    out_d = nc.dram_tensor("out", [N, LSH], FP32, kind="ExternalOutput").ap()

    with tile.TileContext(nc) as tc:
        with (
            tc.tile_pool(name="const", bufs=1) as cpool,
            tc.tile_pool(name="raws", bufs=5) as rpool,
            tc.tile_pool(name="a1p", bufs=2) as a1pool,
            tc.tile_pool(name="work", bufs=2) as wpool,
            tc.tile_pool(name="spsum", bufs=1, space="PSUM") as spool,
            tc.tile_pool(name="apsum", bufs=1, space="PSUM") as apool,
        ):
            # acc rows n=0..15 in cols [0:1250]; cols [1280:1536] of the same
            # 3 banks are the keep-warm filler target (start=False writes
            # never touch the accumulation's has_written bits)
            acc_t = apool.tile([N, ACC_PAD], FP32, tag="acc")
            acc = acc_t[:, 0:LSH]

            # PE warm-up: operand memset on-chip (no DMA dependency); junk
            # lands in acc and is cleared by the real start=True later.
            warm = cpool.tile([P, 512], DT16, tag="warm")
            nc.vector.memset(warm, 0.001)
            for _ in range(N_WARM):
                nc.tensor.matmul(
                    acc[:, 0:512], warm[:, 0:N], warm, start=True, stop=True
                )

            # constants: hot prefix (embT, vwoh, attnT rows 0-511) first,
            # then the rest; one queue, Q7-generated descriptors
            cst = cpool.tile([P, C_W], DT16, tag="cst")
            nc.gpsimd.dma_start(out=cst[:, 0:C_HOT], in_=cst_d[:, 0:C_HOT])
            embT = cst[:, C_EMB0 : C_EMB0 + LSH]
            vwoh = cst[:, C_VW0 : C_VW0 + NTILE * N]
            attnT = cst[:, C_ATT0 : C_ATT0 + R]

            # --- stream DMAs, ALL triggered up-front (distinct slots: the
            # GpSimd FIFO never blocks on a buffer-free wait here).  Slot
            # reuse happens only for pair 5 + remainder, emitted mid-loop.
            pairs = []
            for p in range(NPAIR - 1):
                rawp = rpool.tile([P, 2 * W], DT16, tag="raw")
                r0 = ROW0[2 * p]
                if p == 0:
                    nc.gpsimd.dma_start(
                        out=rawp[:, 0 : 2 * LSH], in_=raw_d[r0 : r0 + P, 0 : 2 * LSH]
                    )
                    nc.gpsimd.dma_start(
                        out=rawp[:, 2 * LSH : W], in_=raw_d[r0 : r0 + P, 2 * LSH : W]
                    )
                    # cold constants ride after the first tile's halves
                    nc.gpsimd.dma_start(
                        out=cst[:, C_HOT:C_W], in_=cst_d[:, C_HOT:C_W]
                    )
                else:
                    nc.gpsimd.dma_start(out=rawp[:, 0:W], in_=raw_d[r0 : r0 + P])
                nc.gpsimd.dma_start(
                    out=rawp[:, W : 2 * W], in_=raw_d[r0 + P : r0 + 2 * P]
                )
                pairs.append(rawp)

            def emit_S(t, s_ps, chunks, rows=P):
                base = chunks[0][0]
                for c0, w in chunks:
                    nc.tensor.matmul(
                        s_ps[:rows, c0 : c0 + w],
                        attnT[:, ROW0[t] : ROW0[t] + rows],
                        embT[:, c0 - base : c0 - base + w],
                        start=True,
                        stop=True,
                    )

            def emit_acc(pt, u, v, *, first, last):
                for half, t in ((0, u), (1, v)):
                    for c0, w in CHUNKS:
                        nc.tensor.matmul(
                            acc[:, c0 : c0 + w],
                            vwoh[:, t * N : (t + 1) * N],
                            pt[:, half * LSH + c0 : half * LSH + c0 + w],
                            start=(first and half == 0),
                            stop=last,
                        )

            pending = None  # (pt, s_sb, a2, u, v) of the previous pair

            for p in range(NPAIR):
                u, v = 2 * p, 2 * p + 1
                if p < NPAIR - 1:
                    rawp = pairs[p]
                else:
                    # pair 5 reuses slot 0 (pair 0 consumed long before)
                    rawp = rpool.tile([P, 2 * W], DT16, tag="raw")
                    r0 = ROW0[2 * p]
                    nc.gpsimd.dma_start(out=rawp[:, 0:W], in_=raw_d[r0 : r0 + P])
                    nc.gpsimd.dma_start(
                        out=rawp[:, W : 2 * W], in_=raw_d[r0 + P : r0 + 2 * P]
                    )

                s_ps = spool.tile([P, 2 * LSH], FP32, tag="s")  # 5 banks
                emit_S(u, s_ps, CHUNKS)
                # evacuate banks 0-1 while the PE writes tile v (banks 2-4)
                s_sb = wpool.tile([P, 2 * LSH], DT16, tag="ssb")
                nc.scalar.copy(out=s_sb[:, 0:1024], in_=s_ps[:, 0:1024])
                emit_S(v, s_ps, CHUNKS1)
                nc.scalar.copy(
                    out=s_sb[:, 1024 : 2 * LSH], in_=s_ps[:, 1024 : 2 * LSH]
                )

                # previous pair: decay-mul first (its ACT finished last
                # period - no DVE stall), then this pair's folds
                if pending is not None:
                    ppt, pssb, pa2, pu, pv = pending
                    nc.vector.tensor_mul(out=ppt, in0=pa2, in1=pssb)
                    emit_acc(ppt, pu, pv, first=(p == 1), last=False)
                    # keep-warm fillers: depend on ppt so they stay in-flow
                    for _ in range(N_FILL):
                        nc.tensor.matmul(
                            acc_t[:, 1280:1536],
                            warm[:, 0:N],
                            ppt[:, 0:256],
                            start=False,
                            stop=True,
                            skip_group_check=True,
                        )

                a1 = a1pool.tile([P, 2 * 2 * LSH], DT16, tag="a1")
                a2 = wpool.tile([P, 2 * LSH], DT16, tag="a2")
                pt = wpool.tile([P, 2 * LSH], DT16, tag="pt")
                if p == 0:
                    # tile 0's fold starts after its first half-DMA
                    nc.vector.tensor_add(
                        out=a1[:, 0:LSH], in0=rawp[:, 0:LSH], in1=rawp[:, LSH : 2 * LSH]
                    )
                    nc.vector.tensor_add(
                        out=a1[:, LSH : 2 * LSH],
                        in0=rawp[:, 2 * LSH : 3 * LSH],
                        in1=rawp[:, 3 * LSH : 4 * LSH],
                    )
                    nc.vector.tensor_add(
                        out=a1[:, 2 * LSH : 4 * LSH],
                        in0=rawp[:, W : W + 2 * LSH],
                        in1=rawp[:, W + 2 * LSH : 2 * W],
                    )
                    a1r = a1.rearrange("p (t c) -> p t c", t=2)
                    a2r = a2.rearrange("p (t c) -> p t c", t=2)
                    nc.vector.tensor_add(
                        out=a2r, in0=a1r[:, :, 0:LSH], in1=a1r[:, :, LSH : 2 * LSH]
                    )
                else:
                    rr = rawp.rearrange("p (t h c) -> p t h c", t=2, h=2)
                    a1r = a1.rearrange("p (t c) -> p t c", t=2)
                    nc.vector.tensor_add(out=a1r, in0=rr[:, :, 0], in1=rr[:, :, 1])
                    a2r = a2.rearrange("p (t c) -> p t c", t=2)
                    nc.vector.tensor_add(
                        out=a2r, in0=a1r[:, :, 0:LSH], in1=a1r[:, :, LSH : 2 * LSH]
                    )
                pending = (pt, s_sb, a2, u, v)

                if p == 2:
                    # remainder DMAs reuse slot 1 (pair 1 consumed by now)
                    rem = rpool.tile([P, 2 * W], DT16, tag="raw")
                    rv = rem[:, 0:W].rearrange("p (d l) -> p d l", d=D)[0:64]
                    rd3 = raw_d.rearrange("r (d l) -> r d l", d=D)
                    rr_ = slice(ROW0[12], ROW0[12] + 64)
                    nc.gpsimd.dma_start(out=rv[:, 0:2], in_=rd3[rr_, 0:2])
                    nc.gpsimd.dma_start(
                        out=rv[:, 2:4, 0:HALF], in_=rd3[rr_, 2:4, 0:HALF]
                    )
                    nc.gpsimd.dma_start(
                        out=rv[:, 2:4, HALF:LSH], in_=rd3[rr_, 2:4, HALF:LSH]
                    )

            # ---- remainder tile (64 rows)
            t = 12
            s_ps = spool.tile([P, 2 * LSH], FP32, tag="s")
            emit_S(t, s_ps, CHUNKS, rows=64)

            # pair 5 epilogue ops
            ppt, pssb, pa2, pu, pv = pending
            nc.vector.tensor_mul(out=ppt, in0=pa2, in1=pssb)
            emit_acc(ppt, pu, pv, first=False, last=False)

            s_sb_f = wpool.tile([P, 2 * LSH], DT16, tag="ssb")
            s_sb = s_sb_f[:, 0:LSH]
            nc.scalar.copy(out=s_sb, in_=s_ps[:, 0:LSH])

            a1_f = a1pool.tile([P, 2 * 2 * LSH], DT16, tag="a1")
            a2_f = wpool.tile([P, 2 * LSH], DT16, tag="a2")
            pt_f = wpool.tile([P, 2 * LSH], DT16, tag="pt")
            a1 = a1_f[:, 0 : 2 * LSH]
            a2 = a2_f[:, 0:LSH]
            pt = pt_f[:, 0:LSH]
            raw = rem
            nc.vector.tensor_add(
                out=a1[:, 0:LSH], in0=raw[:, 0:LSH], in1=raw[:, LSH : 2 * LSH]
            )
            for h0, h1 in ((0, HALF), (HALF, LSH)):
                nc.vector.tensor_add(
                    out=a1[:, LSH + h0 : LSH + h1],
                    in0=raw[:, 2 * LSH + h0 : 2 * LSH + h1],
                    in1=raw[:, 3 * LSH + h0 : 3 * LSH + h1],
                )
                nc.vector.tensor_add(
                    out=a2[:, h0:h1],
                    in0=a1[:, h0:h1],
                    in1=a1[:, LSH + h0 : LSH + h1],
                )
                nc.vector.tensor_mul(
                    out=pt[:, h0:h1], in0=a2[:, h0:h1], in1=s_sb[:, h0:h1]
                )

            # final acc matmuls interleaved per chunk with the PSUM copy and
            # a per-chunk output DMA on the sync HWDGE ring
            out_sb = cpool.tile([N, LSH], FP32, tag="out_sb")
            for c0, w in CHUNKS:
                nc.tensor.matmul(
                    acc[:, c0 : c0 + w],
                    vwoh[:, t * N : (t + 1) * N],
                    pt[:, c0 : c0 + w],
                    start=False,
                    stop=True,
                )
                nc.vector.tensor_copy(
                    out=out_sb[:, c0 : c0 + w], in_=acc[:, c0 : c0 + w]
                )
                nc.sync.dma_start(
                    out=out_d[:, c0 : c0 + w], in_=out_sb[:, c0 : c0 + w]
                )

    nc.compile()
    return nc


def _get_nc():
    if "nc" not in _NC_CACHE:
        _NC_CACHE["nc"] = _build_nc()
    return _NC_CACHE["nc"]


def _prep_in_maps(self_attn, self_delta, emb_table, value_w):
    self_attn = np.asarray(self_attn, dtype=np.float32)
    self_delta = np.asarray(self_delta, dtype=np.float32)
    emb_table = np.asarray(emb_table, dtype=np.float32)
    value_w = np.asarray(value_w, dtype=np.float32)

    # int8 quantization of the delta stream (device DMA widens to bf16)
    qs = 4.0 * float(self_delta.std()) / 127.0
    raw_q = np.clip(np.rint(self_delta * (1.0 / qs)), -127, 127).astype(np.int8)

    embT_full = emb_table[1 : L + 1].T.astype(BF16)  # [EMB, L]
    attnT = self_attn.transpose(2, 0, 1).reshape(EMB, R).astype(BF16)

    # vwoh[p, t*N + j] = qs * vw[m(r)] * (n(r) == j),  r = ROW0[t] + p
    vwoh = np.zeros((P, NTILE * N), dtype=BF16)
    for t in range(NTILE):
        pmax = 64 if t == NTILE - 1 else P
        for p in range(pmax):
            r = ROW0[t] + p
            vwoh[p, t * N + (r // M)] = qs * value_w[r % M]

    in_maps = []
    for c in range(NCORES):
        lo = c * LSH
        raw_c = np.empty((R, W), dtype=np.int8)
        raw_c.reshape(N, M, D, LSH)[...] = raw_q[:, :, lo : lo + LSH, :].transpose(
            0, 1, 3, 2
        )
        cst = np.empty((P, C_W), dtype=BF16)
        cst[:, C_EMB0 : C_EMB0 + LSH] = embT_full[:, lo : lo + LSH]
        cst[:, C_VW0 : C_VW0 + NTILE * N] = vwoh
        cst[:, C_ATT0 : C_ATT0 + R] = attnT
        in_maps.append({"raw": raw_c, "cst": cst})
    return in_maps


def _run(inputs, **spmd_kwargs):
    in_maps = _prep_in_maps(
        inputs["self_attn"], inputs["self_delta"], inputs["emb_table"], inputs["value_w"]
    )
    res = run_bass_kernel_spmd(
        _get_nc(), in_maps, core_ids=list(range(NCORES)), **spmd_kwargs
    )
    out = np.concatenate([r["out"] for r in res.results], axis=1)  # [N, L]
    return out, res


def kernel(**inputs) -> np.ndarray:
    out, _ = _run(inputs)
    return out


# revision 12
# speedup vs baseline: 1.2572x; 1.0490x over previous
"""Trainium2 Bass kernel: fused bmm+decay+reduce attention scorer.

Computes, for full inputs
    self_attn  [N=16, M=100, EMB=128] f32
    self_delta [N=16, M=100, L=10000, D=4] f32
    emb_table  [L+1=10001, EMB=128] f32
    value_w    [M=100] f32
the output
    out[n, l] = sum_m value_w[m] * (sum_d self_delta[n,m,l,d]) * (emb_table[1+l] . self_attn[n,m])
of shape [16, 10000] f32 (matches the reference jnp einsum chain).

Sharding: the candidate/location axis L is split 8 ways (1250 locations per
core); every core handles all 16 batch rows for its location range.

v6.  Measured foundations (traces of v0=83us .. v5=74us):
  - self_delta staged host-side as int8 (quant step 4*sigma/127 folded into
    vwoh) and widened to bf16 inside the SWDGE cast-DMA; the cast stream
    measured 425 GB/s on the SBUF-AXI write side (HBM side halved, not
    binding).  ~16.8 MB of SBUF writes -> ~39.5us of stream.
  - DVE op cost = (58 + FD/2)/0.96GHz + ~100ns drain (v5 exact).  The DVE
    fold/mul chain (~38us serial) is the END-TO-END binding resource, so:
    tiles processed in PAIRS sharing one slot (one drain per op covers two
    tiles); the 64-row remainder runs EARLY, in the ramp where the DVE
    idles; the final pair is split per-tile so only one tile's fold+mul
    (~2.4us) trails the stream's last byte.
  - GpSimd tensor ops block the DVE cycle-for-cycle (shared SBUF port) —
    GpSimd only generates DMA descriptors here.  Its NX is strict FIFO, so
    ALL stream triggers are emitted up-front into distinct buffers (no
    buffer-reuse waits anywhere on the queue).
  - PE HAM clock gate needs >~62-90% duty per 3413ns window for 2.4 GHz.
    Warm-up MMs bridge the preamble; filler MMs whose moving operand is the
    previous pair's pt (a real dependency — the scheduler cannot hoist
    them) pad steady-state duty.  Fillers write the padding tail of the
    acc banks with start=False, leaving accumulation bits untouched.
  - The remainder's acc matmuls carry the accumulation-clearing start=True
    (they run first), also wiping the warm-up junk; the last pair's carry
    stop=True.  attnT is stored column-rotated (remainder rows first) so
    the hot constant prefix covers the remainder + pair 0.
  - ACTIVATE evacuation split [0:1024]/[1024:2500] (bank-disjoint from the
    in-flight S matmuls) shortens the S->ACT->S serial chain.
  - Per-chunk output copy + sync-HWDGE DMA interleaved with the final accs.
Fixed framework overhead per execution: ~6.5us preamble + ~7.5us trailing
next-iteration preamble (253 serial semaphore resets).
"""

import ml_dtypes
import numpy as np

import concourse.mybir as mybir
import concourse.tile as tile
from concourse import bacc
from concourse.bass_utils import run_bass_kernel_spmd

BF16 = ml_dtypes.bfloat16

N, M, L, EMB, D = 16, 100, 10000, 128, 4
NCORES = 8
LSH = L // NCORES  # 1250 locations per core
R = N * M  # 1600 flattened (n, m) rows
P = 128
NTILE = 13  # 12 full 128-row tiles (6 pairs) + one 64-row remainder
ROW0 = [t * P for t in range(NTILE - 1)] + [R - 64]
NPAIR = 6
CHUNKS = [(0, 512), (512, 512), (1024, 226)]  # acc banks / tile-0-of-pair S
CHUNKS1 = [(1250, 286), (1536, 512), (2048, 452)]  # tile-1-of-pair S chunks
W = D * LSH  # 5000 columns per tile in the raw stream
ACC_PAD = 1536  # acc PSUM tile padded to 3 full banks; tail = filler target
DT16 = mybir.dt.bfloat16
INT8 = mybir.dt.int8
FP32 = mybir.dt.float32
N_WARM = 6  # warm-up matmuls (N=512) bridging preamble -> first S matmul
N_FILL = 4  # keep-warm filler matmuls (N=256) per pair
# const concat layout (bf16 cols): embT | vwoh | attnT(col-rotated, rem first)
C_EMB0 = 0
C_VW0 = LSH
C_ATT0 = LSH + NTILE * N
C_W = C_ATT0 + R
C_HOT = C_ATT0 + 64 + 2 * P  # embT + vwoh + attnT cols for remainder + pair 0

_NC_CACHE = {}


def _att0(t):
    # attnT storage column of ROW0[t] (remainder's 64 rows stored first)
    return 64 + ROW0[t] if t < 12 else 0


def _build_nc():
    nc = bacc.Bacc(
        "TRN2", target_bir_lowering=False, debug=False, num_devices=NCORES
    )
    raw_d = nc.dram_tensor("raw", [R, W], INT8, kind="ExternalInput").ap()
    cst_d = nc.dram_tensor("cst", [P, C_W], DT16, kind="ExternalInput").ap()
    out_d = nc.dram_tensor("out", [N, LSH], FP32, kind="ExternalOutput").ap()

    with tile.TileContext(nc) as tc:
        with (
            tc.tile_pool(name="const", bufs=1) as cpool,
            tc.tile_pool(name="raws", bufs=6) as rpool,
            tc.tile_pool(name="remp", bufs=1) as rempool,
            tc.tile_pool(name="a1p", bufs=1) as a1pool,
            tc.tile_pool(name="work", bufs=2) as wpool,
            tc.tile_pool(name="spsum", bufs=1, space="PSUM") as spool,
            tc.tile_pool(name="apsum", bufs=1, space="PSUM") as apool,
        ):
            acc_t = apool.tile([N, ACC_PAD], FP32, tag="acc")
            acc = acc_t[:, 0:LSH]

            # PE warm-up (junk into acc, cleared by the remainder's
            # start=True accumulation later)
            warm = cpool.tile([P, 512], DT16, tag="warm")
            nc.vector.memset(warm, 0.001)
            for _ in range(N_WARM):
                nc.tensor.matmul(
                    acc[:, 0:512], warm[:, 0:N], warm, start=True, stop=True
                )

            # --- all DMA triggers up-front, distinct buffers, no waits ---
            cst = cpool.tile([P, C_W], DT16, tag="cst")
            nc.gpsimd.dma_start(out=cst[:, 0:C_HOT], in_=cst_d[:, 0:C_HOT])
            embT = cst[:, C_EMB0 : C_EMB0 + LSH]
            vwoh = cst[:, C_VW0 : C_VW0 + NTILE * N]
            attnT = cst[:, C_ATT0 : C_ATT0 + R]

            remraw = rempool.tile([P, W], DT16, tag="remraw")
            nc.gpsimd.dma_start(
                out=remraw[0:64], in_=raw_d[ROW0[12] : ROW0[12] + 64]
            )

            pairs = []
            for p in range(NPAIR):
                rawp = rpool.tile([P, 2 * W], DT16, tag="raw")
                r0 = ROW0[2 * p]
                if p == 0:
                    # first tile in halves (folds start earlier), then the
                    # cold constants, then tile 1
                    nc.gpsimd.dma_start(
                        out=rawp[:, 0 : 2 * LSH], in_=raw_d[r0 : r0 + P, 0 : 2 * LSH]
                    )
                    nc.gpsimd.dma_start(
                        out=rawp[:, 2 * LSH : W], in_=raw_d[r0 : r0 + P, 2 * LSH : W]
                    )
                    nc.gpsimd.dma_start(
                        out=cst[:, C_HOT:C_W], in_=cst_d[:, C_HOT:C_W]
                    )
                    nc.gpsimd.dma_start(
                        out=rawp[:, W : 2 * W], in_=raw_d[r0 + P : r0 + 2 * P]
                    )
                elif p < NPAIR - 1:
                    # one trigger per pair: 256 rows folded to 128x(2,5000)
                    nc.gpsimd.dma_start(
                        out=rawp.rearrange("p (t c) -> p t c", t=2),
                        in_=raw_d[r0 : r0 + 2 * P].rearrange(
                            "(t p) c -> p t c", t=2
                        ),
                    )
                else:
                    # last pair split per tile; tile 11 in halves so only a
                    # half-tile of fold work trails the stream
                    nc.gpsimd.dma_start(out=rawp[:, 0:W], in_=raw_d[r0 : r0 + P])
                    nc.gpsimd.dma_start(
                        out=rawp[:, W : W + 2 * LSH],
                        in_=raw_d[r0 + P : r0 + 2 * P, 0 : 2 * LSH],
                    )
                    nc.gpsimd.dma_start(
                        out=rawp[:, W + 2 * LSH : 2 * W],
                        in_=raw_d[r0 + P : r0 + 2 * P, 2 * LSH : W],
                    )
                pairs.append(rawp)

            def emit_S(t, s_ps, chunks, rows=P):
                base = chunks[0][0]
                a0 = _att0(t)
                for c0, w in chunks:
                    nc.tensor.matmul(
                        s_ps[:rows, c0 : c0 + w],
                        attnT[:, a0 : a0 + rows],
                        embT[:, c0 - base : c0 - base + w],
                        start=True,
                        stop=True,
                    )

            def acc_mm(t, pt_slice, *, start, stop, rows=P):
                for c0, w in CHUNKS:
                    nc.tensor.matmul(
                        acc[:, c0 : c0 + w],
                        vwoh[0:rows, t * N : (t + 1) * N],
                        pt_slice[0:rows, c0 : c0 + w],
                        start=start,
                        stop=stop,
                    )

            # ---- remainder tile first: folds/mul run in the ramp where the
            # DVE is otherwise idle; its accs carry the clearing start=True
            s_ps = spool.tile([P, 2 * LSH], FP32, tag="s")
            emit_S(12, s_ps, CHUNKS, rows=64)
            rs_sb = wpool.tile([P, 2 * LSH], DT16, tag="ssb")
            nc.scalar.copy(out=rs_sb[:, 0:LSH], in_=s_ps[:, 0:LSH])

            ra1 = a1pool.tile([P, 2 * 2 * LSH], DT16, tag="a1")
            ra2 = wpool.tile([P, 2 * LSH], DT16, tag="a2")
            rpt = wpool.tile([P, 2 * LSH], DT16, tag="pt")
            nc.vector.tensor_add(
                out=ra1[:, 0 : 2 * LSH],
                in0=remraw[:, 0 : 2 * LSH],
                in1=remraw[:, 2 * LSH : W],
            )
            nc.vector.tensor_add(
                out=ra2[:, 0:LSH], in0=ra1[:, 0:LSH], in1=ra1[:, LSH : 2 * LSH]
            )
            nc.vector.tensor_mul(
                out=rpt[:, 0:LSH], in0=ra2[:, 0:LSH], in1=rs_sb[:, 0:LSH]
            )
            # contract over 64 partitions only: rows 64-127 of the remainder
            # buffer are uninitialized SBUF and must not touch the matmul
            acc_mm(12, rpt, start=True, stop=False, rows=64)

            pending = None  # (ssb, a2, pt, u, v) of the previous pair

            for p in range(NPAIR):
                rawp = pairs[p]
                u, v = 2 * p, 2 * p + 1
                s_ps = spool.tile([P, 2 * LSH], FP32, tag="s")
                emit_S(u, s_ps, CHUNKS)
                s_sb = wpool.tile([P, 2 * LSH], DT16, tag="ssb")
                nc.scalar.copy(out=s_sb[:, 0:1024], in_=s_ps[:, 0:1024])
                emit_S(v, s_ps, CHUNKS1)
                nc.scalar.copy(
                    out=s_sb[:, 1024 : 2 * LSH], in_=s_ps[:, 1024 : 2 * LSH]
                )

                if pending is not None:
                    pssb, pa2, ppt, pu, pv = pending
                    nc.vector.tensor_mul(out=ppt, in0=pa2, in1=pssb)
                    acc_mm(pu, ppt[:, 0:LSH], start=False, stop=False)
                    acc_mm(pv, ppt[:, LSH : 2 * LSH], start=False, stop=False)
                    for _ in range(N_FILL):
                        nc.tensor.matmul(
                            acc_t[:, 1280:1536],
                            warm[:, 0:N],
                            ppt[:, 0:256],
                            start=False,
                            stop=True,
                            skip_group_check=True,
                        )

                a1 = a1pool.tile([P, 2 * 2 * LSH], DT16, tag="a1")
                a2 = wpool.tile([P, 2 * LSH], DT16, tag="a2")
                pt = wpool.tile([P, 2 * LSH], DT16, tag="pt")
                if p == 0:
                    nc.vector.tensor_add(
                        out=a1[:, 0:LSH], in0=rawp[:, 0:LSH], in1=rawp[:, LSH : 2 * LSH]
                    )
                    nc.vector.tensor_add(
                        out=a1[:, LSH : 2 * LSH],
                        in0=rawp[:, 2 * LSH : 3 * LSH],
                        in1=rawp[:, 3 * LSH : 4 * LSH],
                    )
                    nc.vector.tensor_add(
                        out=a1[:, 2 * LSH : 4 * LSH],
                        in0=rawp[:, W : W + 2 * LSH],
                        in1=rawp[:, W + 2 * LSH : 2 * W],
                    )
                    a1r = a1.rearrange("p (t c) -> p t c", t=2)
                    a2r = a2.rearrange("p (t c) -> p t c", t=2)
                    nc.vector.tensor_add(
                        out=a2r, in0=a1r[:, :, 0:LSH], in1=a1r[:, :, LSH : 2 * LSH]
                    )
                    pending = (s_sb, a2, pt, u, v)
                elif p < NPAIR - 1:
                    rr = rawp.rearrange("p (t h c) -> p t h c", t=2, h=2)
                    a1r = a1.rearrange("p (t c) -> p t c", t=2)
                    nc.vector.tensor_add(out=a1r, in0=rr[:, :, 0], in1=rr[:, :, 1])
                    a2r = a2.rearrange("p (t c) -> p t c", t=2)
                    nc.vector.tensor_add(
                        out=a2r, in0=a1r[:, :, 0:LSH], in1=a1r[:, :, LSH : 2 * LSH]
                    )
                    pending = (s_sb, a2, pt, u, v)
                else:
                    # last pair: per-tile chain, shortest trail after the
                    # stream's final byte
                    nc.vector.tensor_add(
                        out=a1[:, 0 : 2 * LSH],
                        in0=rawp[:, 0 : 2 * LSH],
                        in1=rawp[:, 2 * LSH : W],
                    )
                    nc.vector.tensor_add(
                        out=a2[:, 0:LSH], in0=a1[:, 0:LSH], in1=a1[:, LSH : 2 * LSH]
                    )
                    nc.vector.tensor_mul(
                        out=pt[:, 0:LSH], in0=a2[:, 0:LSH], in1=s_sb[:, 0:LSH]
                    )
                    # tile 11: d0+d1 after its first half lands
                    nc.vector.tensor_add(
                        out=a1[:, 2 * LSH : 3 * LSH],
                        in0=rawp[:, W : W + LSH],
                        in1=rawp[:, W + LSH : W + 2 * LSH],
                    )
                    nc.vector.tensor_add(
                        out=a1[:, 3 * LSH : 4 * LSH],
                        in0=rawp[:, W + 2 * LSH : W + 3 * LSH],
                        in1=rawp[:, W + 3 * LSH : 2 * W],
                    )
                    nc.vector.tensor_add(
                        out=a2[:, LSH : 2 * LSH],
                        in0=a1[:, 2 * LSH : 3 * LSH],
                        in1=a1[:, 3 * LSH : 4 * LSH],
                    )
                    nc.vector.tensor_mul(
                        out=pt[:, LSH : 2 * LSH],
                        in0=a2[:, LSH : 2 * LSH],
                        in1=s_sb[:, LSH : 2 * LSH],
                    )
                    last_pt = pt

            # (pair 4's mul+accs were emitted inside iteration 5's pending
            # block; nothing further is owed here)

            # final pair's accs interleaved per chunk with the PSUM copy and
            # a per-chunk output DMA on the sync HWDGE ring
            out_sb = cpool.tile([N, LSH], FP32, tag="out_sb")
            for c0, w in CHUNKS:
                nc.tensor.matmul(
                    acc[:, c0 : c0 + w],
                    vwoh[:, 10 * N : 11 * N],
                    last_pt[:, c0 : c0 + w],
                    start=False,
                    stop=False,
                )
                nc.tensor.matmul(
                    acc[:, c0 : c0 + w],
                    vwoh[:, 11 * N : 12 * N],
                    last_pt[:, LSH + c0 : LSH + c0 + w],
                    start=False,
                    stop=True,
                )
                nc.vector.tensor_copy(
                    out=out_sb[:, c0 : c0 + w], in_=acc[:, c0 : c0 + w]
                )
                nc.sync.dma_start(
                    out=out_d[:, c0 : c0 + w], in_=out_sb[:, c0 : c0 + w]
                )

    nc.compile()
    return nc


def _get_nc():
    if "nc" not in _NC_CACHE:
        _NC_CACHE["nc"] = _build_nc()
    return _NC_CACHE["nc"]


def _prep_in_maps(self_attn, self_delta, emb_table, value_w):
    self_attn = np.asarray(self_attn, dtype=np.float32)
    self_delta = np.asarray(self_delta, dtype=np.float32)
    emb_table = np.asarray(emb_table, dtype=np.float32)
    value_w = np.asarray(value_w, dtype=np.float32)

    # int8 quantization of the delta stream (device DMA widens to bf16)
    qs = 4.0 * float(self_delta.std()) / 127.0
    raw_q = np.clip(np.rint(self_delta * (1.0 / qs)), -127, 127).astype(np.int8)

    embT_full = emb_table[1 : L + 1].T.astype(BF16)  # [EMB, L]
    attnT = self_attn.transpose(2, 0, 1).reshape(EMB, R).astype(BF16)
    # column-rotate: remainder rows (1536..1599) first
    attnT_rot = np.concatenate([attnT[:, R - 64 :], attnT[:, : R - 64]], axis=1)

    # vwoh[p, t*N + j] = qs * vw[m(r)] * (n(r) == j),  r = ROW0[t] + p
    vwoh = np.zeros((P, NTILE * N), dtype=BF16)
    for t in range(NTILE):
        pmax = 64 if t == NTILE - 1 else P
        for p in range(pmax):
            r = ROW0[t] + p
            vwoh[p, t * N + (r // M)] = qs * value_w[r % M]

    in_maps = []
    for c in range(NCORES):
        lo = c * LSH
        raw_c = np.empty((R, W), dtype=np.int8)
        raw_c.reshape(N, M, D, LSH)[...] = raw_q[:, :, lo : lo + LSH, :].transpose(
            0, 1, 3, 2
        )
        cst = np.empty((P, C_W), dtype=BF16)
        cst[:, C_EMB0 : C_EMB0 + LSH] = embT_full[:, lo : lo + LSH]
        cst[:, C_VW0 : C_VW0 + NTILE * N] = vwoh
        cst[:, C_ATT0 : C_ATT0 + R] = attnT_rot
        in_maps.append({"raw": raw_c, "cst": cst})
    return in_maps


def _run(inputs, **spmd_kwargs):
    in_maps = _prep_in_maps(
        inputs["self_attn"], inputs["self_delta"], inputs["emb_table"], inputs["value_w"]
    )
    res = run_bass_kernel_spmd(
        _get_nc(), in_maps, core_ids=list(range(NCORES)), **spmd_kwargs
    )
    out = np.concatenate([r["out"] for r in res.results], axis=1)  # [N, L]
    return out, res


def kernel(**inputs) -> np.ndarray:
    out, _ = _run(inputs)
    return out
